# revision 8
# baseline (speedup 1.0000x reference)
"""GCNConv (manual) Trainium2 Bass kernel, 8-core SPMD.

Strategy (memory-regime, gnn_message_passing):
  out = D^-1/2 (A+I) D^-1/2 X W + bias, computed as (Â X) W by linearity:
  - Host: shard destination nodes contiguously across 8 cores; sort each
    core's incoming edges by destination group (64 dsts per group); append
    one self-edge per node; factorize the symmetric norm as
    xs = D^-1/2 X folded into the streamed source rows and D^-1/2[dst]
    applied after aggregation. The source rows xs[src[e]] are laid out as
    an affine, partition-major bf16 edge stream (standard graph prepacking
    for accelerators without fast data-dependent gather).
  - Device (per core): for each 64-dst group, accumulate on the
    TensorEngine with the one-hot as the stationary operand (64-column
    weight loads):  agg[d, k] += onehot[e, d]^T @ xs_tile[e, k],
    where onehot[e, d] = (iota_d == code[e]) is built 8 tiles per fused
    VectorEngine compare. Then per group: transpose agg via the PE,
    transform  po[d, c] = aggT[k, d]^T @ W[k, c], scale by deg^-1/2[dst]
    (ScalarEngine, fused with PSUM evacuation), add bias if nonzero, and
    DMA out. Everything is affine DMA + dense matmul.
"""

import sys

for _p in ("/opt/trn_rl_repo",):
    if _p not in sys.path:
        sys.path.insert(0, _p)

import numpy as np
import ml_dtypes

import concourse.bacc as bacc
import concourse.bass as bass
import concourse.mybir as mybir
import concourse.tile as tile
from concourse.bass_utils import run_bass_kernel_spmd

N_NODES = 50000
N_EDGES = 800000
C = 128
N_CORES = 8
P = 128                      # partitions / tile height (edges per tile)
GW = 64                      # dst-group width
ROWS_PER_CORE = N_NODES // N_CORES          # 6250
G = (ROWS_PER_CORE + GW - 1) // GW          # 98 groups per core
ROWS_PAD = ((ROWS_PER_CORE + P - 1) // P) * P   # 6272 (=49 pair-blocks)
NPAIR = ROWS_PAD // P                        # 49
OH_BATCH = 16                # tiles per fused one-hot compare

bf16 = mybir.dt.bfloat16
f32 = mybir.dt.float32

_cache = {}


def _preprocess(x, edge_index, weight, bias):
    """Host-side sharding/layout. Returns (in_maps, tiles_per_group, has_bias)."""
    x = np.asarray(x, dtype=np.float32)
    edge_index = np.asarray(edge_index)
    weight = np.asarray(weight, dtype=np.float32)
    bias = np.asarray(bias, dtype=np.float32)

    src_all = edge_index[0].astype(np.int64)
    dst_all = edge_index[1].astype(np.int64)

    deg = np.bincount(dst_all, minlength=N_NODES).astype(np.float64) + 1.0
    dis = (1.0 / np.sqrt(deg)).astype(np.float32)   # deg >= 1 always

    xs_bf = (x * dis[:, None]).astype(ml_dtypes.bfloat16)   # D^-1/2 X

    core_of = dst_all // ROWS_PER_CORE
    order = np.argsort(core_of, kind="stable")
    src_s, dst_s = src_all[order], dst_all[order]
    core_bounds = np.searchsorted(core_of[order], np.arange(N_CORES + 1))

    per_core = []
    for c in range(N_CORES):
        lo, hi = core_bounds[c], core_bounds[c + 1]
        s, d = src_s[lo:hi], dst_s[lo:hi]
        dl = d - c * ROWS_PER_CORE
        dl_self = np.arange(ROWS_PER_CORE, dtype=np.int64)
        s_self = dl_self + c * ROWS_PER_CORE
        s = np.concatenate([s, s_self])
        dl = np.concatenate([dl, dl_self])
        g = dl // GW
        o = np.argsort(g, kind="stable")
        s, dl, g = s[o], dl[o], g[o]
        gb = np.searchsorted(g, np.arange(G + 1))
        per_core.append((s, dl, gb))

    cnt = np.array([[pc[2][gi + 1] - pc[2][gi] for gi in range(G)]
                    for pc in per_core])          # [cores, G]
    tiles_per_group = ((cnt.max(axis=0) + P - 1) // P).astype(np.int64)  # [G]
    TT = int(tiles_per_group.sum())

    in_maps = []
    for c in range(N_CORES):
        s, dl, gb = per_core[c]
        src_stream = np.zeros(TT * P, dtype=np.int64)
        code = np.full(TT * P, 384.0, dtype=np.float32)   # 384 => no match
        pos = 0
        for gi in range(G):
            e0, e1 = gb[gi], gb[gi + 1]
            n = e1 - e0
            src_stream[pos:pos + n] = s[e0:e1]
            code[pos:pos + n] = (dl[e0:e1] % GW).astype(np.float32)
            pos += int(tiles_per_group[gi]) * P
        xsrc = xs_bf[src_stream].reshape(TT, P, C).transpose(1, 0, 2)  # [128,TT,C]
        code_pm = code.reshape(TT, P).T                                # [128,TT]

        # dis arranged to match paired-group output tiles [128, NPAIR]
        dis_local = np.zeros((P, NPAIR), dtype=np.float32)
        dl_rows = np.arange(ROWS_PER_CORE, dtype=np.int64)
        dis_local[dl_rows % P, dl_rows // P] = dis[c * ROWS_PER_CORE + dl_rows]

        in_maps.append({
            "xsrc": np.ascontiguousarray(xsrc),
            "code": np.ascontiguousarray(code_pm).astype(ml_dtypes.bfloat16),
            "w": weight.astype(ml_dtypes.bfloat16),
            "dis_local": dis_local,
            "bias_tile": np.tile(bias[None, :], (P, 1)).astype(np.float32),
            "iota": np.tile(np.arange(GW, dtype=np.float32)[None, :],
                            (P, OH_BATCH)).astype(ml_dtypes.bfloat16),
            "ident": np.eye(P, dtype=np.float32).astype(ml_dtypes.bfloat16),
        })
    has_bias = bool(np.any(bias != 0.0))
    return in_maps, tiles_per_group, has_bias


def _build(tiles_per_group, has_bias):
    TT = int(tiles_per_group.sum())
    nc = bacc.Bacc("TRN2", target_bir_lowering=False, debug=False)

    xsrc_d = nc.dram_tensor("xsrc", [P, TT, C], bf16, kind="ExternalInput")
    code_d = nc.dram_tensor("code", [P, TT], bf16, kind="ExternalInput")
    w_d = nc.dram_tensor("w", [C, C], bf16, kind="ExternalInput")
    disl_d = nc.dram_tensor("dis_local", [P, NPAIR], f32, kind="ExternalInput")
    bias_d = nc.dram_tensor("bias_tile", [P, C], f32, kind="ExternalInput")
    iota_d = nc.dram_tensor("iota", [P, OH_BATCH * GW], bf16, kind="ExternalInput")
    ident_d = nc.dram_tensor("ident", [P, P], bf16, kind="ExternalInput")
    out_d = nc.dram_tensor("out", [ROWS_PAD, C], f32, kind="ExternalOutput")
    out_v = out_d.rearrange("(g p) c -> p g c", p=P)   # [128, NPAIR, C]

    # ramped chunk sizes: start tiny so the first matmuls fire immediately
    chunks = []
    t0 = 0
    for sz in (16, 16, 32, 64):
        if t0 >= TT:
            break
        chunks.append((t0, min(t0 + sz, TT)))
        t0 = chunks[-1][1]
    while t0 < TT:
        chunks.append((t0, min(t0 + 128, TT)))
        t0 += 128
    chunk_of = np.zeros(TT, dtype=int)
    for ci, (c0, c1) in enumerate(chunks):
        chunk_of[c0:c1] = ci

    gstart = np.concatenate([[0], np.cumsum(tiles_per_group)]).astype(int)

    with tile.TileContext(nc) as tc:
        with (
            tc.tile_pool(name="const", bufs=1) as constp,
            tc.tile_pool(name="stream", bufs=3) as streamp,
            tc.tile_pool(name="meta", bufs=1) as metap,
            tc.tile_pool(name="work", bufs=8) as workp,
            tc.tile_pool(name="aggp", bufs=4) as aggp,
            tc.tile_pool(name="outp", bufs=3) as outp,
            tc.tile_pool(name="psum_agg", bufs=3, space="PSUM") as psum_agg,
            tc.tile_pool(name="psum_tr", bufs=2, space="PSUM") as psum_tr,
            tc.tile_pool(name="psum_out", bufs=2, space="PSUM") as psum_out,
        ):
            w_t = constp.tile([C, C], bf16)
            iota_t = constp.tile([P, OH_BATCH, GW], bf16)
            disl_t = constp.tile([P, NPAIR], f32)
            bias_t = constp.tile([P, C], f32)
            ident_t = constp.tile([P, P], bf16)
            nc.sync.dma_start(w_t[:], w_d[:])
            nc.sync.dma_start(
                iota_t[:], iota_d.rearrange("p (b q) -> p b q", b=OH_BATCH)[:])
            nc.sync.dma_start(disl_t[:], disl_d[:])
            nc.sync.dma_start(bias_t[:], bias_d[:])
            nc.sync.dma_start(ident_t[:], ident_d[:])
            code_t = metap.tile([P, TT], bf16)
            nc.sync.dma_start(code_t[:], code_d[:])

            chunk_tiles = {}

            def get_chunk(ci):
                if ci not in chunk_tiles:
                    c0, c1 = chunks[ci]
                    st = streamp.tile([P, 128, C], bf16, tag="stream")
                    nc.sync.dma_start(st[:, : c1 - c0, :], xsrc_d[:, c0:c1, :])
                    chunk_tiles[ci] = st
                return chunk_tiles[ci]

            oh_tiles = {}

            def get_onehot(t):
                if t not in oh_tiles:
                    b0 = (t // OH_BATCH) * OH_BATCH
                    b1 = min(b0 + OH_BATCH, TT)
                    nb = b1 - b0
                    oh = workp.tile([P, OH_BATCH, GW], bf16, tag="onehot")
                    nc.vector.tensor_tensor(
                        oh[:, :nb, :],
                        iota_t[:, :nb, :],
                        code_t[:, b0:b1][:, :, None].broadcast_to([P, nb, GW]),
                        op=mybir.AluOpType.is_equal,
                    )
                    for tt in range(b0, b1):
                        oh_tiles[tt] = (oh, tt - b0)
                return oh_tiles[t]

            def emit_dance(gj, agg2):
                # agg2[d2, k] (two 64-groups stacked) -> transpose ->
                # aggT2[k, d2] -> po2[d2, c] -> scale -> out
                agg_sb = aggp.tile([P, C], bf16, tag="agg_sb")
                nc.scalar.activation(
                    agg_sb[:], agg2[:], mybir.ActivationFunctionType.Copy)
                trp = psum_tr.tile([P, P], bf16)
                nc.tensor.transpose(trp[:], agg_sb[:], ident_t[:])
                aggT = aggp.tile([P, P], bf16, tag="aggT")
                nc.vector.tensor_copy(aggT[:], trp[:])
                po = psum_out.tile([P, C], f32)
                nc.tensor.matmul(po[:], aggT[:], w_t[:], start=True, stop=True)
                ot = outp.tile([P, C], f32, tag="out")
                nc.scalar.activation(
                    ot[:], po[:],
                    mybir.ActivationFunctionType.Copy,
                    scale=disl_t[:, gj : gj + 1],
                )
                if has_bias:
                    nc.vector.tensor_add(ot[:], ot[:], bias_t[:])
                nc.sync.dma_start(out_v[:, gj, :], ot[:])

            pending = []
            for gj in range(NPAIR):
                agg2 = psum_agg.tile([P, C], f32)
                for h in range(2):
                    gi = gj * 2 + h
                    ts, te = gstart[gi], gstart[gi + 1]
                    for t in range(ts, te):
                        ci = int(chunk_of[t])
                        st = get_chunk(ci)
                        oh, slot = get_onehot(t)
                        nc.tensor.matmul(
                            agg2[h * GW : (h + 1) * GW, :],
                            oh[:, slot, :],                 # lhsT [e, d]
                            st[:, t - chunks[ci][0], :],    # rhs  [e, k]
                            start=(t == ts),
                            stop=(t == te - 1),
                        )
                pending.append((gj, agg2))
                if len(pending) > 1:
                    emit_dance(*pending.pop(0))
            for item in pending:
                emit_dance(*item)

    nc.compile()
    return nc


def kernel(x, edge_index, weight, bias):
    key = (np.asarray(x).shape, np.asarray(edge_index).shape)
    in_maps, tpg, has_bias = _preprocess(x, edge_index, weight, bias)
    ck = key + (tuple(tpg), has_bias)
    if ck not in _cache:
        _cache[ck] = _build(tpg, has_bias)
    nc = _cache[ck]
    res = run_bass_kernel_spmd(nc, in_maps, core_ids=list(range(N_CORES)))
    out = np.empty((N_NODES, C), dtype=np.float32)
    for c in range(N_CORES):
        out[c * ROWS_PER_CORE : (c + 1) * ROWS_PER_CORE] = (
            res.results[c]["out"][:ROWS_PER_CORE]
        )
    return out


# revision 9
# speedup vs baseline: 1.0632x; 1.0632x over previous
"""GCNConv (manual) Trainium2 Bass kernel, 8-core SPMD.

Strategy (memory-regime, gnn_message_passing):
  out = D^-1/2 (A+I) D^-1/2 X W + bias, computed as (Â X) W by linearity:
  - Host: shard destination nodes contiguously across 8 cores; sort each
    core's incoming edges by destination group (64 dsts per group); append
    one self-edge per node; factorize the symmetric norm as
    xs = D^-1/2 X folded into the streamed source rows and D^-1/2[dst]
    applied after aggregation. The source rows xs[src[e]] are laid out as
    an affine, partition-major bf16 edge stream (standard graph prepacking
    for accelerators without fast data-dependent gather).
  - Device (per core): for each 64-dst group, accumulate on the
    TensorEngine with the one-hot as the stationary operand (64-column
    weight loads):  agg[d, k] += onehot[e, d]^T @ xs_tile[e, k],
    where onehot[e, d] = (iota_d == code[e]) is built 8 tiles per fused
    VectorEngine compare. Then per group: transpose agg via the PE,
    transform  po[d, c] = aggT[k, d]^T @ W[k, c], scale by deg^-1/2[dst]
    (ScalarEngine, fused with PSUM evacuation), add bias if nonzero, and
    DMA out. Everything is affine DMA + dense matmul.
"""

import sys

for _p in ("/opt/trn_rl_repo",):
    if _p not in sys.path:
        sys.path.insert(0, _p)

import numpy as np
import ml_dtypes

import concourse.bacc as bacc
import concourse.bass as bass
import concourse.mybir as mybir
import concourse.tile as tile
from concourse.bass_utils import run_bass_kernel_spmd

N_NODES = 50000
N_EDGES = 800000
C = 128
N_CORES = 8
P = 128                      # partitions / tile height (edges per tile)
GW = 64                      # dst-group width
ROWS_PER_CORE = N_NODES // N_CORES          # 6250
G = (ROWS_PER_CORE + GW - 1) // GW          # 98 groups per core
ROWS_PAD = ((ROWS_PER_CORE + P - 1) // P) * P   # 6272 (=49 pair-blocks)
NPAIR = ROWS_PAD // P                        # 49
OH_BATCH = 8                 # tiles per fused one-hot compare

bf16 = mybir.dt.bfloat16
f32 = mybir.dt.float32

_cache = {}


def _preprocess(x, edge_index, weight, bias):
    """Host-side sharding/layout. Returns (in_maps, tiles_per_group, has_bias)."""
    x = np.asarray(x, dtype=np.float32)
    edge_index = np.asarray(edge_index)
    weight = np.asarray(weight, dtype=np.float32)
    bias = np.asarray(bias, dtype=np.float32)

    src_all = edge_index[0].astype(np.int64)
    dst_all = edge_index[1].astype(np.int64)

    deg = np.bincount(dst_all, minlength=N_NODES).astype(np.float64) + 1.0
    dis = (1.0 / np.sqrt(deg)).astype(np.float32)   # deg >= 1 always

    xs_bf = (x * dis[:, None]).astype(ml_dtypes.bfloat16)   # D^-1/2 X

    core_of = dst_all // ROWS_PER_CORE
    order = np.argsort(core_of, kind="stable")
    src_s, dst_s = src_all[order], dst_all[order]
    core_bounds = np.searchsorted(core_of[order], np.arange(N_CORES + 1))

    per_core = []
    for c in range(N_CORES):
        lo, hi = core_bounds[c], core_bounds[c + 1]
        s, d = src_s[lo:hi], dst_s[lo:hi]
        dl = d - c * ROWS_PER_CORE
        dl_self = np.arange(ROWS_PER_CORE, dtype=np.int64)
        s_self = dl_self + c * ROWS_PER_CORE
        s = np.concatenate([s, s_self])
        dl = np.concatenate([dl, dl_self])
        g = dl // GW
        o = np.argsort(g, kind="stable")
        s, dl, g = s[o], dl[o], g[o]
        gb = np.searchsorted(g, np.arange(G + 1))
        per_core.append((s, dl, gb))

    cnt = np.array([[pc[2][gi + 1] - pc[2][gi] for gi in range(G)]
                    for pc in per_core])          # [cores, G]
    tiles_per_group = ((cnt.max(axis=0) + P - 1) // P).astype(np.int64)  # [G]
    TT = int(tiles_per_group.sum())

    in_maps = []
    for c in range(N_CORES):
        s, dl, gb = per_core[c]
        src_stream = np.zeros(TT * P, dtype=np.int64)
        code = np.full(TT * P, 384.0, dtype=np.float32)   # 384 => no match
        pos = 0
        for gi in range(G):
            e0, e1 = gb[gi], gb[gi + 1]
            n = e1 - e0
            src_stream[pos:pos + n] = s[e0:e1]
            code[pos:pos + n] = (dl[e0:e1] % GW).astype(np.float32)
            pos += int(tiles_per_group[gi]) * P
        xsrc = xs_bf[src_stream].reshape(TT, P, C).transpose(1, 0, 2)  # [128,TT,C]
        code_pm = code.reshape(TT, P).T                                # [128,TT]

        # dis arranged to match paired-group output tiles [128, NPAIR]
        dis_local = np.zeros((P, NPAIR), dtype=np.float32)
        dl_rows = np.arange(ROWS_PER_CORE, dtype=np.int64)
        dis_local[dl_rows % P, dl_rows // P] = dis[c * ROWS_PER_CORE + dl_rows]

        in_maps.append({
            "xsrc": np.ascontiguousarray(xsrc),
            "code": np.ascontiguousarray(code_pm).astype(ml_dtypes.bfloat16),
            "w": weight.astype(ml_dtypes.bfloat16),
            "dis_local": dis_local,
            "bias_tile": np.tile(bias[None, :], (P, 1)).astype(np.float32),
            "iota": np.tile(np.arange(GW, dtype=np.float32)[None, :],
                            (P, OH_BATCH)).astype(ml_dtypes.bfloat16),
            "ident": np.eye(P, dtype=np.float32).astype(ml_dtypes.bfloat16),
        })
    has_bias = bool(np.any(bias != 0.0))
    return in_maps, tiles_per_group, has_bias


def _build(tiles_per_group, has_bias):
    TT = int(tiles_per_group.sum())
    nc = bacc.Bacc("TRN2", target_bir_lowering=False, debug=False)

    xsrc_d = nc.dram_tensor("xsrc", [P, TT, C], bf16, kind="ExternalInput")
    code_d = nc.dram_tensor("code", [P, TT], bf16, kind="ExternalInput")
    w_d = nc.dram_tensor("w", [C, C], bf16, kind="ExternalInput")
    disl_d = nc.dram_tensor("dis_local", [P, NPAIR], f32, kind="ExternalInput")
    bias_d = nc.dram_tensor("bias_tile", [P, C], f32, kind="ExternalInput")
    iota_d = nc.dram_tensor("iota", [P, OH_BATCH * GW], bf16, kind="ExternalInput")
    ident_d = nc.dram_tensor("ident", [P, P], bf16, kind="ExternalInput")
    out_d = nc.dram_tensor("out", [ROWS_PAD, C], f32, kind="ExternalOutput")
    out_v = out_d.rearrange("(g p) c -> p g c", p=P)   # [128, NPAIR, C]

    # ramped chunk sizes: start tiny so the first matmuls fire immediately
    chunks = []
    t0 = 0
    for sz in (32, 96):
        if t0 >= TT:
            break
        chunks.append((t0, min(t0 + sz, TT)))
        t0 = chunks[-1][1]
    while t0 < TT:
        chunks.append((t0, min(t0 + 128, TT)))
        t0 += 128
    chunk_of = np.zeros(TT, dtype=int)
    for ci, (c0, c1) in enumerate(chunks):
        chunk_of[c0:c1] = ci

    gstart = np.concatenate([[0], np.cumsum(tiles_per_group)]).astype(int)

    with tile.TileContext(nc) as tc:
        with (
            tc.tile_pool(name="const", bufs=1) as constp,
            tc.tile_pool(name="stream", bufs=3) as streamp,
            tc.tile_pool(name="meta", bufs=1) as metap,
            tc.tile_pool(name="work", bufs=8) as workp,
            tc.tile_pool(name="aggp", bufs=4) as aggp,
            tc.tile_pool(name="outp", bufs=3) as outp,
            tc.tile_pool(name="psum_agg", bufs=3, space="PSUM") as psum_agg,
            tc.tile_pool(name="psum_tr", bufs=2, space="PSUM") as psum_tr,
            tc.tile_pool(name="psum_out", bufs=2, space="PSUM") as psum_out,
        ):
            w_t = constp.tile([C, C], bf16)
            iota_t = constp.tile([P, OH_BATCH, GW], bf16)
            disl_t = constp.tile([P, NPAIR], f32)
            bias_t = constp.tile([P, C], f32)
            ident_t = constp.tile([P, P], bf16)
            nc.sync.dma_start(w_t[:], w_d[:])
            nc.sync.dma_start(
                iota_t[:], iota_d.rearrange("p (b q) -> p b q", b=OH_BATCH)[:])
            nc.sync.dma_start(disl_t[:], disl_d[:])
            nc.sync.dma_start(bias_t[:], bias_d[:])
            nc.sync.dma_start(ident_t[:], ident_d[:])
            code_t = metap.tile([P, TT], bf16)
            nc.sync.dma_start(code_t[:], code_d[:])

            chunk_tiles = {}

            def get_chunk(ci):
                if ci not in chunk_tiles:
                    c0, c1 = chunks[ci]
                    st = streamp.tile([P, 128, C], bf16, tag="stream")
                    nc.sync.dma_start(st[:, : c1 - c0, :], xsrc_d[:, c0:c1, :])
                    chunk_tiles[ci] = st
                return chunk_tiles[ci]

            oh_tiles = {}

            def get_onehot(t):
                if t not in oh_tiles:
                    b0 = (t // OH_BATCH) * OH_BATCH
                    b1 = min(b0 + OH_BATCH, TT)
                    nb = b1 - b0
                    oh = workp.tile([P, OH_BATCH, GW], bf16, tag="onehot")
                    nc.vector.tensor_tensor(
                        oh[:, :nb, :],
                        iota_t[:, :nb, :],
                        code_t[:, b0:b1][:, :, None].broadcast_to([P, nb, GW]),
                        op=mybir.AluOpType.is_equal,
                    )
                    for tt in range(b0, b1):
                        oh_tiles[tt] = (oh, tt - b0)
                return oh_tiles[t]

            def emit_dance(gj, agg2):
                # agg2[d2, k] (two 64-groups stacked) -> transpose ->
                # aggT2[k, d2] -> po2[d2, c] -> scale -> out
                agg_sb = aggp.tile([P, C], bf16, tag="agg_sb")
                nc.scalar.activation(
                    agg_sb[:], agg2[:], mybir.ActivationFunctionType.Copy)
                trp = psum_tr.tile([P, P], bf16)
                nc.tensor.transpose(trp[:], agg_sb[:], ident_t[:])
                aggT = aggp.tile([P, P], bf16, tag="aggT")
                nc.vector.tensor_copy(aggT[:], trp[:])
                po = psum_out.tile([P, C], f32)
                nc.tensor.matmul(po[:], aggT[:], w_t[:], start=True, stop=True)
                ot = outp.tile([P, C], f32, tag="out")
                nc.scalar.activation(
                    ot[:], po[:],
                    mybir.ActivationFunctionType.Copy,
                    scale=disl_t[:, gj : gj + 1],
                )
                if has_bias:
                    nc.vector.tensor_add(ot[:], ot[:], bias_t[:])
                nc.sync.dma_start(out_v[:, gj, :], ot[:])

            pending = []
            for gj in range(NPAIR):
                agg2 = psum_agg.tile([P, C], f32)
                for h in range(2):
                    gi = gj * 2 + h
                    ts, te = gstart[gi], gstart[gi + 1]
                    for t in range(ts, te):
                        ci = int(chunk_of[t])
                        st = get_chunk(ci)
                        oh, slot = get_onehot(t)
                        nc.tensor.matmul(
                            agg2[h * GW : (h + 1) * GW, :],
                            oh[:, slot, :],                 # lhsT [e, d]
                            st[:, t - chunks[ci][0], :],    # rhs  [e, k]
                            start=(t == ts),
                            stop=(t == te - 1),
                        )
                pending.append((gj, agg2))
                if len(pending) > 1:
                    emit_dance(*pending.pop(0))
            for item in pending:
                emit_dance(*item)

    nc.compile()
    return nc


def kernel(x, edge_index, weight, bias):
    key = (np.asarray(x).shape, np.asarray(edge_index).shape)
    in_maps, tpg, has_bias = _preprocess(x, edge_index, weight, bias)
    ck = key + (tuple(tpg), has_bias)
    if ck not in _cache:
        _cache[ck] = _build(tpg, has_bias)
    nc = _cache[ck]
    res = run_bass_kernel_spmd(nc, in_maps, core_ids=list(range(N_CORES)))
    out = np.empty((N_NODES, C), dtype=np.float32)
    for c in range(N_CORES):
        out[c * ROWS_PER_CORE : (c + 1) * ROWS_PER_CORE] = (
            res.results[c]["out"][:ROWS_PER_CORE]
        )
    return out
